# revision 27
# baseline (speedup 1.0000x reference)
"""Trainium2 Bass kernel for nn_Bert_lattice (FLAT lattice transformer).

Model: B=2,S=256,H=8,D=32,T=256,FF=1024,L=2, four-way relative-position
lattice fusion + 2 transformer layers (no out-proj, double-relu FFN).

Algebraic restructuring:
  * rel = relu(concat(pe[dss],pe[dse],pe[des],pe[dee]) @ W_fus + b_fus)
    -> precompute P_m = pe @ W_fus[mT:(m+1)T] (4 tables [513,256]) so
    rel[b,i,j] = relu(P0[dss]+P1[dse]+P2[des]+P3[dee]+b_fus): a 4-row
    gather + adds instead of a 68.7 GFLOP matmul.
  * BD[b,h,i,j] = (q+v)[b,i,h,:] . (rel[b,i,j,:] @ Wr + br)[h,:]
    -> g[i,h,t] = sum_d Wr[t,h*D+d]*(q+v)[i,h,d];
       BD[i,h,j] = sum_t g[i,h,t]*rel[i,j,t] + const(i,h).
    The const(i,h) (br term) is j-independent and cancels in softmax.
  * Activations kept transposed [feature, token]; LayerNorm reductions
    over features run on the PE via ones-matmuls; per-token stats are
    broadcast back across partitions via rank-1 matmuls.

Sharding: 8 cores; core c owns b=c//4, query rows [64*(c%4), +64).
rel shard (64 x 2 x 128 x 256 bf16) stays SBUF-resident across both
layers. Layer boundary: AllGather of the 64-token output shard within
each group of 4 cores (per-b groups). Host does layout-only prep
(transposes, gather-index/mask precompute, bf16 casts).
"""

import sys

sys.path.insert(0, "/opt/trn_rl_repo")

import numpy as np
import ml_dtypes

BF16 = ml_dtypes.bfloat16
FP8 = ml_dtypes.float8_e4m3

B, S, H, D = 2, 256, 8, 32
T = H * D          # 256
FF = 4 * T         # 1024
MAXSEP = 256
NTAB = 2 * MAXSEP + 1   # 513 rows in the pe table
NROW = 512         # rows kept per fused table (d = 1..512; d=0 unused)
L = 2
EPS = 1e-5
NC = 8
IPC = B * S // NC  # 64 query rows per core
NBATCH = 16        # gather batches per core
IPB = IPC // NBATCH  # 4 i's per gather batch


def build_nc(debug=False):
    from concourse import bacc, tile, mybir

    nc = bacc.Bacc("TRN2", target_bir_lowering=False, debug=False, num_devices=NC)

    F32 = mybir.dt.float32
    BF = mybir.dt.bfloat16
    F8 = mybir.dt.float8e4
    I16 = mybir.dt.int16

    def inp(name, shape, dt=F32):
        return nc.dram_tensor(name, shape, dt, kind="ExternalInput")

    xT_d = inp("xT", [T, S], BF)
    residT_d = inp("residT", [T, IPC])
    peT_d = inp("peT", [T, NTAB], BF)
    wfus_d = inp("wfus", [4 * T, T], BF)
    bfus_d = inp("bfus", [1, T])
    mask_d = inp("maskrow", [1, S])
    os_d = inp("osmat", [S, S], F8)       # flipped one-hot for ps_j
    oe_d = inp("oemat", [S, S], F8)       # flipped one-hot for pe_j
    woff_d = inp("woff", [IPC, 2], mybir.dt.int32)  # ps_i+1, pe_i+1
    wq_d = inp("wq", [L, T, T], BF)
    wk_d = inp("wk", [L, T, T], BF)
    wv_d = inp("wv", [L, T, T], BF)
    wrT_d = inp("wrT", [L, T, T], BF)
    w1_d = inp("w1", [L, T, FF], BF)
    w2_d = inp("w2", [L, FF, T], BF)
    bk_d = inp("bk", [L, T, 1])
    bv_d = inp("bv", [L, 1, T])
    bqu_d = inp("bqu", [L, T, 1])
    bqv_d = inp("bqv", [L, T, 1])
    b1_d = inp("b1", [L, FF, 1])
    b2_d = inp("b2", [L, T, 1])
    outT_d = nc.dram_tensor("outT", [T, IPC], F32, kind="ExternalOutput")

    dbg = {}
    if debug:
        dbg["rel0"] = nc.dram_tensor("dbg_rel0", [128, 2, S], F32, kind="ExternalOutput")
        dbg["score0"] = nc.dram_tensor("dbg_score0", [128, S], F32, kind="ExternalOutput")
        dbg["prob0"] = nc.dram_tensor("dbg_prob0", [128, S], F32, kind="ExternalOutput")
        dbg["attn0"] = nc.dram_tensor("dbg_attn0", [128, 32], F32, kind="ExternalOutput")
        dbg["y1"] = nc.dram_tensor("dbg_y1", [T, IPC], F32, kind="ExternalOutput")
        dbg["out1"] = nc.dram_tensor("dbg_out1", [T, IPC], F32, kind="ExternalOutput")
        dbg["kT1"] = nc.dram_tensor("dbg_kT1", [T, S], F32, kind="ExternalOutput")

    with tile.TileContext(nc) as tc:
        _emit(
            nc, tc, mybir, debug, dbg,
            xT_d=xT_d, residT_d=residT_d, peT_d=peT_d, wfus_d=wfus_d,
            bfus_d=bfus_d, mask_d=mask_d, os_d=os_d, oe_d=oe_d,
            woff_d=woff_d, wq_d=wq_d, wk_d=wk_d,
            wv_d=wv_d, wrT_d=wrT_d, w1_d=w1_d, w2_d=w2_d, bk_d=bk_d,
            bv_d=bv_d, bqu_d=bqu_d, bqv_d=bqv_d, b1_d=b1_d, b2_d=b2_d,
            outT_d=outT_d,
        )
    nc.compile()
    return nc


def _emit(nc, tc, mybir, debug, dbg, **io):
    from concourse import masks, bass
    from contextlib import ExitStack

    F32 = mybir.dt.float32
    BF = mybir.dt.bfloat16
    F8 = mybir.dt.float8e4
    I16 = mybir.dt.int16
    AF = mybir.ActivationFunctionType
    ALU = mybir.AluOpType
    AX = mybir.AxisListType

    es = ExitStack()
    const_p = es.enter_context(tc.tile_pool(name="const", bufs=1))
    dram_p = es.enter_context(tc.tile_pool(name="dramp", bufs=1, space="DRAM"))
    wload_p = es.enter_context(tc.tile_pool(name="wload", bufs=1))
    psum_p = es.enter_context(tc.tile_pool(name="psum", bufs=5, space="PSUM"))
    psrow_p = es.enter_context(tc.tile_pool(name="psrow", bufs=2, space="PSUM"))
    work_p = es.enter_context(tc.tile_pool(name="work", bufs=2))
    add_p = es.enter_context(tc.tile_pool(name="addp", bufs=4))
    rel_p = es.enter_context(tc.tile_pool(name="relp", bufs=1))
    prob_p = es.enter_context(tc.tile_pool(name="probp", bufs=3))
    stat_p = es.enter_context(tc.tile_pool(name="statp", bufs=4))
    pers_p = es.enter_context(tc.tile_pool(name="persp", bufs=1))

    # ---------------- constants ----------------
    ident_bf = const_p.tile([128, 128], BF, tag="ident", name="ident_bf")
    masks.make_identity(nc, ident_bf[:])
    ones_row = const_p.tile([1, 128], F32, tag="onesr", name="ones_row")
    nc.vector.memset(ones_row[:], 1.0)
    ones_col = const_p.tile([128, 1], F32, tag="onesc", name="ones_col")
    nc.vector.memset(ones_col[:], 1.0)

    dma_engs = [nc.sync, nc.gpsimd, nc.scalar]
    _eng_rr = [0]

    def load(p, dram_ap, shape, dt, name, eng=None):
        t = p.tile(shape, dt, tag=name, name=name)
        if eng is None:
            eng = dma_engs[_eng_rr[0] % len(dma_engs)]
            _eng_rr[0] += 1
        eng.dma_start(t[:], dram_ap)
        return t

    col2 = lambda d: d[:].rearrange("(c p) o -> p c o", p=128)  # [2C*128,1] -> [128,C,1]
    chunk = lambda d: d[:].rearrange("(c p) s -> p c s", p=128)

    # phase-0-critical consts first, spread over the three DMA queues
    peT_sb = load(const_p, chunk(io["peT_d"]), [128, 2, NTAB], BF, "peT_sb", nc.sync)
    wfus_sb = load(const_p, chunk(io["wfus_d"]), [128, 8, T], BF, "wfus_sb", nc.gpsimd)
    bfus_sb = load(const_p, io["bfus_d"][:], [1, T], F32, "bfus_sb", nc.scalar)
    os_sb = load(const_p, chunk(io["os_d"]), [128, 2, S], F8, "os_sb", nc.sync)
    oe_sb = load(const_p, chunk(io["oe_d"]), [128, 2, S], F8, "oe_sb", nc.gpsimd)
    woff_sb = load(const_p, io["woff_d"][:], [IPC, 2], mybir.dt.int32, "woff_sb", nc.scalar)
    mask_sb = load(const_p, io["mask_d"][:], [1, S], F32, "mask_sb", nc.scalar)
    xT_sb = load(pers_p, chunk(io["xT_d"]), [128, 2, S], BF, "xT_sb", nc.sync)
    residT_sb = load(pers_p, chunk(io["residT_d"]), [128, 2, IPC], F32, "residT_sb", nc.gpsimd)

    # per-layer weights/biases: loads are emitted sprinkled into phase 1 on the
    # scalar queue (they are only needed at layer 0 start)
    wload_specs = []
    for l in range(L):
        wload_specs += [
            ("wq", l, chunk(io["wq_d"][l]), [128, 2, T], BF),
            ("wk", l, chunk(io["wk_d"][l]), [128, 2, T], BF),
            ("wv", l, chunk(io["wv_d"][l]), [128, 2, T], BF),
            ("wrT", l, chunk(io["wrT_d"][l]), [128, 2, T], BF),
            ("w1", l, chunk(io["w1_d"][l]), [128, 2, FF], BF),
            ("w2", l, chunk(io["w2_d"][l]), [128, 8, T], BF),
            ("bk", l, col2(io["bk_d"][l]), [128, 2, 1], F32),
            ("bv", l, io["bv_d"][l], [1, T], F32),
            ("bqu", l, col2(io["bqu_d"][l]), [128, 2, 1], F32),
            ("bqv", l, col2(io["bqv_d"][l]), [128, 2, 1], F32),
            ("b1", l, col2(io["b1_d"][l]), [128, 8, 1], F32),
            ("b2", l, col2(io["b2_d"][l]), [128, 2, 1], F32),
        ]
    # interleave layer-0 and layer-1 loads so layer 0 is ready first
    order = [x for pair in zip(wload_specs[:12], wload_specs[12:]) for x in pair]
    wload_specs = order
    W = {}

    def emit_next_wload():
        if wload_specs:
            nm, l, ap, shp, dt = wload_specs.pop(0)
            W[nm, l] = load(wload_p, ap, shp, dt, f"{nm}_{l}", nc.scalar)

    # ---------------- phase 0: fused tables P_m = pe @ W_fus[mT:(m+1)T] ----------------
    # Stored pairwise-interleaved by window key so each i needs 2 window DMAs:
    #   pcatA[d-1, 0/1, :] = P0/P1[d] (+bfus into P0)   (key ps_i)
    #   pcatB[d-1, 0/1, :] = P2/P3[d]                   (key pe_i)
    pcatA = dram_p.tile([NROW, 2, T], F8, tag="pcatA", name="pcatA")
    pcatB = dram_p.tile([NROW, 2, T], F8, tag="pcatB", name="pcatB")
    obA = add_p.tile([128, 4, 2, T], F8, tag="p0outA", name="p0oA")
    obB = add_p.tile([128, 4, 2, T], F8, tag="p0outB", name="p0oB")
    for m in range(4):
        ob = obA if m < 2 else obB
        for c in range(4):
            ps = psum_p.tile([128, 512], F32, tag="ps", name=f"p0_{m}_{c}")
            for kc in range(2):
                nc.tensor.matmul(
                    ps[:, :T],
                    peT_sb[:, kc, 1 + c * 128: 1 + (c + 1) * 128],
                    wfus_sb[:, m * 2 + kc, :],
                    start=(kc == 0), stop=(kc == 1 and m != 0),
                )
            if m == 0:
                nc.tensor.matmul(ps[:, :T], ones_row[:], bfus_sb[:],
                                 start=False, stop=True)
            nc.scalar.activation(ob[:, c, m % 2, :], ps[:, :T], AF.Copy)
    nc.sync.dma_start(pcatA[:].rearrange("(c p) m t -> p c m t", p=128), obA[:])
    nc.gpsimd.dma_start(pcatB[:].rearrange("(c p) m t -> p c m t", p=128), obB[:])

    # ---------------- phase 1: window loads + one-hot selection ----------------
    rel_tiles = [rel_p.tile([128, 2, S], BF, tag=f"rel{i}", name=f"rel_{i}") for i in range(IPC)]

    win_p = es.enter_context(tc.tile_pool(name="winp", bufs=6))
    from concourse.tile import add_dep_helper
    dma_hist = {nc.sync.engine: [], nc.gpsimd.engine: []}
    for i in range(IPC):
        rows = {}
        for k, eng in ((0, nc.sync), (1, nc.gpsimd)):
            hist = dma_hist[eng.engine]
            tmp = eng.alloc_register(f"woffr_{i}_{k}")
            ld = eng.reg_load(tmp, woff_sb[i:i + 1, k:k + 1])
            if len(hist) >= 12:
                add_dep_helper(ld.ins, hist[-12].ins, sync=False)
            rows[k] = eng.snap(tmp, donate=True, min_val=0, max_val=NROW - S)
        # winA[p, c, mslot, t]: row c*128+p of the 256-row window, table pair
        # A loaded then pair B accumulated on top via SDMA CCE add.
        winA = win_p.tile([128, 2, 2, S], F8, tag="winA", name=f"wA_{i}")
        dmi = nc.sync.dma_start(
            winA[:], pcatA[:][bass.ds(rows[0], S), :, :].rearrange("(c p) m t -> p c m t", p=128))
        dma_hist[nc.sync.engine].append(dmi)
        dmi = nc.gpsimd.dma_start(
            winA[:], pcatB[:][bass.ds(rows[1], S), :, :].rearrange("(c p) m t -> p c m t", p=128),
            accum_op=ALU.add)
        dma_hist[nc.gpsimd.engine].append(dmi)
        # one-hot selection on PE: rel_pre[t, j] = sum_u w[u, t] * onehot[u, j]
        ps = psum_p.tile([128, 512], F32, tag="ps", name=f"rp_{i}")
        nmm = 0
        for tpo in range(2):
            for mslot, oh in ((0, os_sb), (1, oe_sb)):
                nmm += 1
                nc.tensor.matmul(
                    ps[:, tpo * S:(tpo + 1) * S],
                    winA[:, :, mslot, tpo * 128:(tpo + 1) * 128],
                    oh[:],
                    start=(nmm in (1, 3)), stop=(nmm in (2, 4)),
                    perf_mode=mybir.MatmulPerfMode.DoubleRow,
                )
        if i % 2 == 0:
            nc.scalar.activation(rel_tiles[i][:], ps[:], AF.Relu)
        else:
            nc.vector.tensor_scalar(rel_tiles[i][:], ps[:], 0.0, None, ALU.max)
        emit_next_wload()

    if debug:
        r0 = add_p.tile([128, 2, S], F32, tag="dbgr", name="dbgrel")
        nc.vector.tensor_copy(r0[:], rel_tiles[0][:])
        nc.sync.dma_start(dbg["rel0"][:], r0[:])

    # persistent block-diag buffers (zeros survive across layers)
    g_blk = pers_p.tile([128, 2, 16 * IPB * 32], BF, tag="gblk", name="gblk")
    nc.vector.memset(g_blk[:], 0.0)
    qud = pers_p.tile([128, 2, IPC * 8], BF, tag="qud", name="qud")
    nc.vector.memset(qud[:], 0.0)

    # ---------------- phase 2: transformer layers ----------------
    curT_own = residT_sb  # [128, 2, IPC] fp32: own 64 tokens

    def layer_norm_T(src, name):
        mean_ps = psrow_p.tile([1, IPC], F32, tag="psr", name=f"mn_{name}")
        for c in range(2):
            nc.tensor.matmul(mean_ps[:], ones_col[:], src[:, c, :], start=(c == 0), stop=(c == 1))
        mean_sb = stat_p.tile([1, IPC], F32, tag="strow", name=f"mns_{name}")
        nc.vector.tensor_scalar_mul(mean_sb[:], mean_ps[:], 1.0 / T)
        mb_ps = psum_p.tile([128, 512], F32, tag="ps", name=f"mb_{name}")
        nc.tensor.matmul(mb_ps[:, :IPC], ones_row[:], mean_sb[:], start=True, stop=True)
        ym = work_p.tile([128, 2, IPC], F32, tag="ym", name=f"ym_{name}")
        ysq = work_p.tile([128, IPC], F32, tag="ysq", name=f"ysq_{name}")
        var_ps = psrow_p.tile([1, IPC], F32, tag="psr", name=f"vr_{name}")
        for c in range(2):
            nc.vector.tensor_sub(ym[:, c, :], src[:, c, :], mb_ps[:, :IPC])
        for c in range(2):
            nc.vector.tensor_mul(ysq[:], ym[:, c, :], ym[:, c, :])
            nc.tensor.matmul(var_ps[:], ones_col[:], ysq[:], start=(c == 0), stop=(c == 1))
        var_sb = stat_p.tile([1, IPC], F32, tag="strow", name=f"vrs_{name}")
        nc.vector.tensor_scalar(var_sb[:], var_ps[:], 1.0 / T, EPS, ALU.mult, ALU.add)
        rstd = stat_p.tile([1, IPC], F32, tag="strow", name=f"rs_{name}")
        nc.vector.reciprocal(rstd[:], var_sb[:])
        nc.scalar.activation(rstd[:], rstd[:], AF.Sqrt)
        rb_ps = psum_p.tile([128, 512], F32, tag="ps", name=f"rb_{name}")
        nc.tensor.matmul(rb_ps[:, :IPC], ones_row[:], rstd[:], start=True, stop=True)
        out = work_p.tile([128, 2, IPC], F32, tag=f"lnout_{name}", name=f"lno_{name}")
        for c in range(2):
            nc.vector.tensor_mul(out[:, c, :], ym[:, c, :], rb_ps[:, :IPC])
        return out

    for l in range(L):
        ownT_bf = work_p.tile([128, 2, IPC], BF, tag="ownbf", name=f"ownbf_{l}")
        nc.scalar.activation(ownT_bf[:], curT_own[:], AF.Copy)
        if l == 0:
            # token c of feature-chunk fc lives at xT_sb[:, fc, c]
            rhs_k = lambda c: xT_sb[:, c, :]
            stat_v = lambda c, jc: xT_sb[:, c, jc * 128:(jc + 1) * 128]
        else:
            rhs_k = lambda c: nxt[:, c, :]
            stat_v = lambda c, jc: nxt[:, c, jc * 128:(jc + 1) * 128]

        wq_sb, wk_sb, wv_sb = W["wq", l], W["wk", l], W["wv", l]
        wrT_sb, w1_sb, w2_sb = W["wrT", l], W["w1", l], W["w2", l]
        bk_sb, bv_sb, bqu_sb = W["bk", l], W["bv", l], W["bqu", l]
        bqv_sb, b1_sb, b2_sb = W["bqv", l], W["b1", l], W["b2", l]

        # ---- qu_T / qv_T [128, 2, IPC] bf16 ----
        quT = work_p.tile([128, 2, IPC], BF, tag="quT", name=f"quT_{l}")
        qvT = work_p.tile([128, 2, IPC], BF, tag="qvT", name=f"qvT_{l}")
        for po in range(2):
            ps = psum_p.tile([128, 512], F32, tag="ps", name=f"qps_{l}_{po}")
            for c in range(2):
                nc.tensor.matmul(ps[:, :IPC], wslice(wq_sb, c, po), ownT_bf[:, c, :], start=(c == 0), stop=(c == 1))
            nc.scalar.activation(quT[:, po, :], ps[:, :IPC], AF.Identity, bias=bqu_sb[:, po, :])
            nc.scalar.activation(qvT[:, po, :], ps[:, :IPC], AF.Identity, bias=bqv_sb[:, po, :])
        for h in range(H):
            hc, hp = divmod(h * D, 128)
            dstq = qud[:, hc, :].rearrange("p (i h) -> p i h", h=8)
            nc.vector.tensor_copy(dstq[hp:hp + D, :, h], quT[hp:hp + D, hc, :])

        # ---- gT -> block-diag g_blk ----
        for h in range(H):
            hc, hp = divmod(h * D, 128)
            for tp in range(2):
                ps = psum_p.tile([128, 512], F32, tag="ps", name=f"gps_{l}_{h}_{tp}")
                nc.tensor.matmul(
                    ps[:, :IPC], wrT_sb[hp:hp + D, hc, tp * 128:(tp + 1) * 128],
                    qvT[hp:hp + D, hc, :], start=True, stop=True,
                    tile_position=(hp, 0),
                )
                srcv = ps[:, :IPC].rearrange("p (s i) -> p s i", i=IPB)
                dstv = g_blk[:, tp, :].rearrange("p (s i c) -> p s i c", i=IPB, c=32)
                for ip in range(IPB):
                    nc.vector.tensor_copy(dstv[:, :, ip, 8 * ip + h], srcv[:, :, ip])

        # ---- k_T [128, 2, S] bf16 (needs all gathered tokens) ----
        kT = work_p.tile([128, 2, S], BF, tag="kT", name=f"kT_{l}")
        for po in range(2):
            ps = psum_p.tile([128, 512], F32, tag="ps", name=f"kps_{l}_{po}")
            for c in range(2):
                nc.tensor.matmul(ps[:, :S], wslice(wk_sb, c, po), rhs_k(c), start=(c == 0), stop=(c == 1))
            nc.scalar.activation(kT[:, po, :], ps[:, :S], AF.Identity, bias=bk_sb[:, po, :])

        # ---- val [128, 2(jc), T] bf16 ----
        val = work_p.tile([128, 2, T], BF, tag="val", name=f"val_{l}")
        for jc in range(2):
            ps = psum_p.tile([128, 512], F32, tag="ps", name=f"vps_{l}_{jc}")
            for c in range(2):
                nc.tensor.matmul(ps[:, :T], stat_v(c, jc), wv_sb[:, c, :], start=(c == 0), stop=False)
            nc.tensor.matmul(ps[:, :T], ones_row[:], bv_sb[:], start=False, stop=True)
            nc.scalar.activation(val[:, jc, :], ps[:, :T], AF.Copy)

        # ---- per-group score / softmax / attention ----
        yT = work_p.tile([128, 2, IPC], F32, tag="yT", name=f"yT_{l}")
        for g in range(4):
            score = psum_p.tile([128, 512], F32, tag="ps", name=f"sc_{l}_{g}")
            for c in range(2):
                nc.tensor.matmul(score[:, :S], qud[:, c, g * 128:(g + 1) * 128], kT[:, c, :], start=(c == 0), stop=False, skip_group_check=True)
            nc.tensor.matmul(score[:, :S], ones_row[:], mask_sb[:], start=False, stop=False, skip_group_check=True)
            for sl in range(4):
                for ip in range(IPB):
                    i = 16 * g + 4 * sl + ip
                    blk = (4 * g + sl) * IPB + ip
                    for tcc in range(2):
                        nc.tensor.matmul(
                            score[32 * sl:32 * sl + 32, :S],
                            g_blk[:, tcc, :].rearrange("p (b c) -> p b c", c=32)[:, blk, :],
                            rel_tiles[i][:, tcc, :],
                            start=False, stop=(ip == IPB - 1 and tcc == 1),
                            tile_position=(0, 32 * sl), skip_group_check=True,
                        )
            # softmax over j
            mx = stat_p.tile([128, 1], F32, tag="st", name=f"mx_{l}_{g}")
            nc.vector.tensor_reduce(mx[:], score[:, :S], AX.X, ALU.max, negate=True)
            prob = prob_p.tile([128, S], BF, tag="prob", name=f"pr_{l}_{g}")
            sum_row = stat_p.tile([128, 1], F32, tag="st", name=f"sm_{l}_{g}")
            nc.scalar.activation(prob[:], score[:, :S], AF.Exp, bias=mx[:], accum_out=sum_row[:])
            rcp = stat_p.tile([128, 1], F32, tag="st", name=f"rc_{l}_{g}")
            nc.vector.reciprocal(rcp[:], sum_row[:])
            nc.vector.tensor_scalar_mul(prob[:], prob[:], rcp[:])
            if debug and l == 0 and g == 0:
                scf = add_p.tile([128, S], F32, tag="dbgsc", name="dbgsc")
                nc.vector.tensor_copy(scf[:], score[:, :S])
                nc.sync.dma_start(dbg["score0"][:], scf[:])
                prf = add_p.tile([128, S], F32, tag="dbgpr", name="dbgpr")
                nc.vector.tensor_copy(prf[:], prob[:])
                nc.sync.dma_start(dbg["prob0"][:], prf[:])
            # prob^T and attention
            attn_ps = psum_p.tile([128, 512], F32, tag="ps", name=f"at_{l}_{g}")
            for jc in range(2):
                pt_ps = psum_p.tile([128, 1024], BF, tag="ps", name=f"pt_{l}_{g}_{jc}")
                nc.tensor.transpose(pt_ps[:, :128], prob[:, jc * 128:(jc + 1) * 128], ident_bf[:])
                pt_sb = prob_p.tile([128, 128], BF, tag="probT", name=f"pts_{l}_{g}_{jc}")
                nc.scalar.activation(pt_sb[:], pt_ps[:, :128], AF.Copy)
                for h in range(H):
                    hm, tau = h % 4, h // 4
                    nc.tensor.matmul(
                        attn_ps[hm * 32:(hm + 1) * 32, tau * 16:(tau + 1) * 16],
                        val[:, jc, h * 32:(h + 1) * 32],
                        pt_sb[:].rearrange("p (q h) -> p q h", h=8)[:, :, h],
                        start=(jc == 0 and tau == 0), stop=(jc == 1 and tau == 1),
                        tile_position=(0, hm * 32), skip_group_check=True,
                    )
            if debug and l == 0 and g == 0:
                atf = add_p.tile([128, 32], F32, tag="dbgat", name="dbgat")
                nc.vector.tensor_copy(atf[:], attn_ps[:, :32])
                nc.sync.dma_start(dbg["attn0"][:], atf[:])
            for fc in range(2):
                nc.vector.tensor_add(
                    yT[:, fc, 16 * g:16 * g + 16],
                    attn_ps[:, fc * 16:(fc + 1) * 16],
                    curT_own[:, fc, 16 * g:16 * g + 16],
                )

        y = layer_norm_T(yT, f"l{l}a")
        if debug and l == 0:
            nc.sync.dma_start(dbg["y1"][:].rearrange("(c p) s -> p c s", p=128), y[:])
        y_bf = work_p.tile([128, 2, IPC], BF, tag="ybf", name=f"ybf_{l}")
        nc.scalar.activation(y_bf[:], y[:], AF.Copy)

        # ---- FFN ----
        h1 = work_p.tile([128, 8, IPC], BF, tag="h1", name=f"h1_{l}")
        for fo in range(8):
            ps = psum_p.tile([128, 512], F32, tag="ps", name=f"h1p_{l}_{fo}")
            for c in range(2):
                nc.tensor.matmul(ps[:, :IPC], w1_sb[:, c, fo * 128:(fo + 1) * 128], y_bf[:, c, :], start=(c == 0), stop=(c == 1))
            nc.scalar.activation(h1[:, fo, :], ps[:, :IPC], AF.Relu, bias=b1_sb[:, fo, :])
        zT = work_p.tile([128, 2, IPC], F32, tag="zT", name=f"zT_{l}")
        for po in range(2):
            ps = psum_p.tile([128, 512], F32, tag="ps", name=f"zp_{l}_{po}")
            for c in range(8):
                nc.tensor.matmul(ps[:, :IPC], w2_sb[:, c, po * 128:(po + 1) * 128], h1[:, c, :], start=(c == 0), stop=(c == 7))
            nc.scalar.activation(zT[:, po, :], ps[:, :IPC], AF.Relu, bias=b2_sb[:, po, :])
        z_res = work_p.tile([128, 2, IPC], F32, tag="zres", name=f"zres_{l}")
        for c in range(2):
            nc.vector.tensor_add(z_res[:, c, :], zT[:, c, :], y[:, c, :])
        outT = layer_norm_T(z_res, f"l{l}b")

        if l == 0:
            if debug:
                nc.sync.dma_start(dbg["out1"][:].rearrange("(c p) s -> p c s", p=128), outT[:])
            # bf16 shard, AllGather, one reload per feature chunk
            sh_bf = work_p.tile([128, 2, IPC], BF, tag="shbf", name="sh_bf")
            nc.scalar.activation(sh_bf[:], outT[:], AF.Copy)
            shard = dram_p.tile([2, 128, IPC], BF, tag="agsh", name="ag_shard")
            nc.sync.dma_start(shard[:].rearrange("f p s -> p f s"), sh_bf[:])
            ag = dram_p.tile([4, 2, 128, IPC], BF, tag="agout", name="ag_out")
            nc.gpsimd.collective_compute(
                "AllGather", mybir.AluOpType.bypass,
                replica_groups=[[0, 1, 2, 3], [4, 5, 6, 7]],
                ins=[shard[:]], outs=[ag[:]],
            )
            nxt = pers_p.tile([128, 2, S], BF, tag="cur2", name="cur2")
            for fc in range(2):
                [nc.sync, nc.scalar][fc].dma_start(
                    nxt[:, fc, :].rearrange("p (q k) -> p q k", k=IPC),
                    ag[:, fc, :, :].rearrange("q p k -> p q k"))
            curT_own = outT
        else:
            nc.sync.dma_start(io["outT_d"][:].rearrange("(c p) s -> p c s", p=128), outT[:])

    es.close()


def wslice(w_sb, c, po):
    """Column slice [po*128, po*128+128) of weight chunk c."""
    return w_sb[:, c, po * 128:(po + 1) * 128]


# ====================== host side ======================

_CACHE = {}


def _get_nc(debug=False):
    key = ("nc", debug)
    if key not in _CACHE:
        _CACHE[key] = build_nc(debug=debug)
    return _CACHE[key]


def make_inputs_for_core(core, x, pos_s, pos_e, real_lengths, lex_num, pe,
                         W_fus, b_fus, Wq, bq, Wk, bk, Wv, bv, Wr, br,
                         u, v, W1, b1, W2, b2):
    b = core // 4
    i0 = (core % 4) * IPC
    xb = np.asarray(x[b], np.float32)          # [S, T]
    ps_b = np.asarray(pos_s[b]).astype(np.int64)
    pe_b = np.asarray(pos_e[b]).astype(np.int64)

    # windows: table m window start row in pcat; one-hot matrices for j-selection.
    iidx = np.arange(i0, i0 + IPC)
    woff = np.empty((IPC, 2), np.int32)
    woff[:, 0] = ps_b[iidx]
    woff[:, 1] = pe_b[iidx]
    osmat = np.zeros((S, S), np.float32)
    osmat[255 - ps_b, np.arange(S)] = 1.0
    oemat = np.zeros((S, S), np.float32)
    oemat[255 - pe_b, np.arange(S)] = 1.0

    keylen = int(real_lengths[b]) + int(lex_num)
    maskrow = np.where(np.arange(S) < keylen, 0.0, -1e15).astype(np.float32)[None, :]

    bf = lambda a: np.ascontiguousarray(np.asarray(a, np.float32)).astype(BF16)
    col = lambda a: np.ascontiguousarray(np.asarray(a, np.float32).reshape(-1, 1))

    uflat = np.asarray(u, np.float32).reshape(L, T)
    vflat = np.asarray(v, np.float32).reshape(L, T)

    return {
        "xT": bf(xb.T),
        "residT": np.ascontiguousarray(xb[i0:i0 + IPC].T),
        "peT": bf(np.asarray(pe, np.float32).T),
        "wfus": bf(W_fus),
        "bfus": np.asarray(b_fus, np.float32).reshape(1, T),
        "maskrow": maskrow,
        "osmat": osmat.astype(FP8), "oemat": oemat.astype(FP8),
        "woff": woff,
        "wq": bf(Wq), "wk": bf(Wk), "wv": bf(Wv),
        "wrT": bf(np.asarray(Wr, np.float32).transpose(0, 2, 1)),
        "w1": bf(W1), "w2": bf(W2),
        "bk": np.asarray(bk, np.float32).reshape(L, T, 1),
        "bv": np.asarray(bv, np.float32).reshape(L, 1, T),
        "bqu": (np.asarray(bq, np.float32) + uflat).reshape(L, T, 1),
        "bqv": (np.asarray(bq, np.float32) + vflat).reshape(L, T, 1),
        "b1": np.asarray(b1, np.float32).reshape(L, FF, 1),
        "b2": np.asarray(b2, np.float32).reshape(L, T, 1),
    }


def kernel(**inputs):
    from concourse.bass_utils import run_bass_kernel_spmd

    nc = _get_nc(debug=False)
    in_maps = [make_inputs_for_core(c, **inputs) for c in range(NC)]
    res = run_bass_kernel_spmd(nc, in_maps, list(range(NC)))
    out = np.empty((B, S, T), np.float32)
    for c in range(NC):
        b = c // 4
        i0 = (c % 4) * IPC
        out[b, i0:i0 + IPC, :] = res.results[c]["outT"].T
    return out



# revision 29
# speedup vs baseline: 1.2735x; 1.2735x over previous
"""Trainium2 Bass kernel for nn_Bert_lattice (FLAT lattice transformer).

Model: B=2,S=256,H=8,D=32,T=256,FF=1024,L=2, four-way relative-position
lattice fusion + 2 transformer layers (no out-proj, double-relu FFN).

Algebraic restructuring:
  * rel = relu(concat(pe[dss],pe[dse],pe[des],pe[dee]) @ W_fus + b_fus)
    -> precompute P_m = pe @ W_fus[mT:(m+1)T] (4 tables [513,256]) so
    rel[b,i,j] = relu(P0[dss]+P1[dse]+P2[des]+P3[dee]+b_fus): a 4-row
    gather + adds instead of a 68.7 GFLOP matmul.
  * BD[b,h,i,j] = (q+v)[b,i,h,:] . (rel[b,i,j,:] @ Wr + br)[h,:]
    -> g[i,h,t] = sum_d Wr[t,h*D+d]*(q+v)[i,h,d];
       BD[i,h,j] = sum_t g[i,h,t]*rel[i,j,t] + const(i,h).
    The const(i,h) (br term) is j-independent and cancels in softmax.
  * Activations kept transposed [feature, token]; LayerNorm reductions
    over features run on the PE via ones-matmuls; per-token stats are
    broadcast back across partitions via rank-1 matmuls.

Sharding: 8 cores; core c owns b=c//4, query rows [64*(c%4), +64).
rel shard (64 x 2 x 128 x 256 bf16) stays SBUF-resident across both
layers. Layer boundary: AllGather of the 64-token output shard within
each group of 4 cores (per-b groups). Host does layout-only prep
(transposes, gather-index/mask precompute, bf16 casts).
"""

import sys

sys.path.insert(0, "/opt/trn_rl_repo")

import numpy as np
import ml_dtypes

BF16 = ml_dtypes.bfloat16
FP8 = ml_dtypes.float8_e4m3

B, S, H, D = 2, 256, 8, 32
T = H * D          # 256
FF = 4 * T         # 1024
MAXSEP = 256
NTAB = 2 * MAXSEP + 1   # 513 rows in the pe table
NROW = 512         # rows kept per fused table (d = 1..512; d=0 unused)
L = 2
EPS = 1e-5
NC = 8
IPC = B * S // NC  # 64 query rows per core
NBATCH = 16        # gather batches per core
IPB = IPC // NBATCH  # 4 i's per gather batch


def build_nc(debug=False):
    from concourse import bacc, tile, mybir

    nc = bacc.Bacc("TRN2", target_bir_lowering=False, debug=False, num_devices=NC)

    F32 = mybir.dt.float32
    BF = mybir.dt.bfloat16
    F8 = mybir.dt.float8e4
    I16 = mybir.dt.int16

    def inp(name, shape, dt=F32):
        return nc.dram_tensor(name, shape, dt, kind="ExternalInput")

    xT_d = inp("xT", [T, S], BF)
    residT_d = inp("residT", [T, IPC])
    peT_d = inp("peT", [T, NTAB], BF)
    wfus_d = inp("wfus", [4 * T, T], BF)
    bfus_d = inp("bfus", [1, T])
    mask_d = inp("maskrow", [1, S])
    os_d = inp("osmat", [S, S], F8)       # flipped one-hot for ps_j
    oe_d = inp("oemat", [S, S], F8)       # flipped one-hot for pe_j
    woff_d = inp("woff", [IPC, 2], mybir.dt.int32)  # ps_i+1, pe_i+1
    wq_d = inp("wq", [L, T, T], BF)
    wk_d = inp("wk", [L, T, T], BF)
    wv_d = inp("wv", [L, T, T], BF)
    wrT_d = inp("wrT", [L, T, T], BF)
    w1_d = inp("w1", [L, T, FF], BF)
    w2_d = inp("w2", [L, FF, T], BF)
    bk_d = inp("bk", [L, T, 1])
    bv_d = inp("bv", [L, 1, T])
    bqu_d = inp("bqu", [L, T, 1])
    bqv_d = inp("bqv", [L, T, 1])
    b1_d = inp("b1", [L, FF, 1])
    b2_d = inp("b2", [L, T, 1])
    outT_d = nc.dram_tensor("outT", [T, IPC], F32, kind="ExternalOutput")

    dbg = {}
    if debug:
        dbg["rel0"] = nc.dram_tensor("dbg_rel0", [128, 2, S], F32, kind="ExternalOutput")
        dbg["score0"] = nc.dram_tensor("dbg_score0", [128, S], F32, kind="ExternalOutput")
        dbg["prob0"] = nc.dram_tensor("dbg_prob0", [128, S], F32, kind="ExternalOutput")
        dbg["attn0"] = nc.dram_tensor("dbg_attn0", [128, 32], F32, kind="ExternalOutput")
        dbg["y1"] = nc.dram_tensor("dbg_y1", [T, IPC], F32, kind="ExternalOutput")
        dbg["out1"] = nc.dram_tensor("dbg_out1", [T, IPC], F32, kind="ExternalOutput")
        dbg["kT1"] = nc.dram_tensor("dbg_kT1", [T, S], F32, kind="ExternalOutput")

    with tile.TileContext(nc) as tc:
        _emit(
            nc, tc, mybir, debug, dbg,
            xT_d=xT_d, residT_d=residT_d, peT_d=peT_d, wfus_d=wfus_d,
            bfus_d=bfus_d, mask_d=mask_d, os_d=os_d, oe_d=oe_d,
            woff_d=woff_d, wq_d=wq_d, wk_d=wk_d,
            wv_d=wv_d, wrT_d=wrT_d, w1_d=w1_d, w2_d=w2_d, bk_d=bk_d,
            bv_d=bv_d, bqu_d=bqu_d, bqv_d=bqv_d, b1_d=b1_d, b2_d=b2_d,
            outT_d=outT_d,
        )
    nc.compile()
    return nc


def _emit(nc, tc, mybir, debug, dbg, **io):
    from concourse import masks, bass
    from contextlib import ExitStack

    F32 = mybir.dt.float32
    BF = mybir.dt.bfloat16
    F8 = mybir.dt.float8e4
    I16 = mybir.dt.int16
    AF = mybir.ActivationFunctionType
    ALU = mybir.AluOpType
    AX = mybir.AxisListType

    es = ExitStack()
    const_p = es.enter_context(tc.tile_pool(name="const", bufs=1))
    dram_p = es.enter_context(tc.tile_pool(name="dramp", bufs=1, space="DRAM"))
    wload_p = es.enter_context(tc.tile_pool(name="wload", bufs=1))
    psum_p = es.enter_context(tc.tile_pool(name="psum", bufs=5, space="PSUM"))
    psrow_p = es.enter_context(tc.tile_pool(name="psrow", bufs=2, space="PSUM"))
    work_p = es.enter_context(tc.tile_pool(name="work", bufs=2))
    add_p = es.enter_context(tc.tile_pool(name="addp", bufs=4))
    rel_p = es.enter_context(tc.tile_pool(name="relp", bufs=1))
    prob_p = es.enter_context(tc.tile_pool(name="probp", bufs=3))
    stat_p = es.enter_context(tc.tile_pool(name="statp", bufs=4))
    pers_p = es.enter_context(tc.tile_pool(name="persp", bufs=1))

    # ---------------- constants ----------------
    ident_bf = const_p.tile([128, 128], BF, tag="ident", name="ident_bf")
    masks.make_identity(nc, ident_bf[:])
    ones_row = const_p.tile([1, 128], F32, tag="onesr", name="ones_row")
    nc.vector.memset(ones_row[:], 1.0)
    ones_col = const_p.tile([128, 1], F32, tag="onesc", name="ones_col")
    nc.vector.memset(ones_col[:], 1.0)

    dma_engs = [nc.sync, nc.gpsimd, nc.scalar]
    _eng_rr = [0]

    def load(p, dram_ap, shape, dt, name, eng=None):
        t = p.tile(shape, dt, tag=name, name=name)
        if eng is None:
            eng = dma_engs[_eng_rr[0] % len(dma_engs)]
            _eng_rr[0] += 1
        eng.dma_start(t[:], dram_ap)
        return t

    col2 = lambda d: d[:].rearrange("(c p) o -> p c o", p=128)  # [2C*128,1] -> [128,C,1]
    chunk = lambda d: d[:].rearrange("(c p) s -> p c s", p=128)

    # phase-0-critical consts first, spread over the three DMA queues
    peT_sb = load(const_p, chunk(io["peT_d"]), [128, 2, NTAB], BF, "peT_sb", nc.sync)
    wfus_sb = load(const_p, chunk(io["wfus_d"]), [128, 8, T], BF, "wfus_sb", nc.gpsimd)
    bfus_sb = load(const_p, io["bfus_d"][:], [1, T], F32, "bfus_sb", nc.scalar)
    os_sb = load(const_p, chunk(io["os_d"]), [128, 2, S], F8, "os_sb", nc.sync)
    oe_sb = load(const_p, chunk(io["oe_d"]), [128, 2, S], F8, "oe_sb", nc.gpsimd)
    woff_sb = load(const_p, io["woff_d"][:], [IPC, 2], mybir.dt.int32, "woff_sb", nc.scalar)
    mask_sb = load(const_p, io["mask_d"][:], [1, S], F32, "mask_sb", nc.scalar)
    xT_sb = load(pers_p, chunk(io["xT_d"]), [128, 2, S], BF, "xT_sb", nc.sync)
    residT_sb = load(pers_p, chunk(io["residT_d"]), [128, 2, IPC], F32, "residT_sb", nc.gpsimd)

    # per-layer weights/biases: loads are emitted sprinkled into phase 1 on the
    # scalar queue (they are only needed at layer 0 start)
    wload_specs = []
    for l in range(L):
        wload_specs += [
            ("wq", l, chunk(io["wq_d"][l]), [128, 2, T], BF),
            ("wk", l, chunk(io["wk_d"][l]), [128, 2, T], BF),
            ("wv", l, chunk(io["wv_d"][l]), [128, 2, T], BF),
            ("wrT", l, chunk(io["wrT_d"][l]), [128, 2, T], BF),
            ("w1", l, chunk(io["w1_d"][l]), [128, 2, FF], BF),
            ("w2", l, chunk(io["w2_d"][l]), [128, 8, T], BF),
            ("bk", l, col2(io["bk_d"][l]), [128, 2, 1], F32),
            ("bv", l, io["bv_d"][l], [1, T], F32),
            ("bqu", l, col2(io["bqu_d"][l]), [128, 2, 1], F32),
            ("bqv", l, col2(io["bqv_d"][l]), [128, 2, 1], F32),
            ("b1", l, col2(io["b1_d"][l]), [128, 8, 1], F32),
            ("b2", l, col2(io["b2_d"][l]), [128, 2, 1], F32),
        ]
    # interleave layer-0 and layer-1 loads so layer 0 is ready first
    order = [x for pair in zip(wload_specs[:12], wload_specs[12:]) for x in pair]
    wload_specs = order
    W = {}

    def emit_next_wload():
        if wload_specs:
            nm, l, ap, shp, dt = wload_specs.pop(0)
            W[nm, l] = load(wload_p, ap, shp, dt, f"{nm}_{l}", nc.scalar)

    # ---------------- phase 0: fused tables P_m = pe @ W_fus[mT:(m+1)T] ----------------
    # Stored pairwise-interleaved by window key so each i needs 2 window DMAs:
    #   pcatA[d-1, 0/1, :] = P0/P1[d] (+bfus into P0)   (key ps_i)
    #   pcatB[d-1, 0/1, :] = P2/P3[d]                   (key pe_i)
    pcatA = dram_p.tile([NROW, 2, T], F8, tag="pcatA", name="pcatA")
    pcatB = dram_p.tile([NROW, 2, T], F8, tag="pcatB", name="pcatB")
    obA = add_p.tile([128, 4, 2, T], F8, tag="p0outA", name="p0oA")
    obB = add_p.tile([128, 4, 2, T], F8, tag="p0outB", name="p0oB")
    for m in range(4):
        ob = obA if m < 2 else obB
        for c in range(4):
            ps = psum_p.tile([128, 512], F32, tag="ps", name=f"p0_{m}_{c}")
            for kc in range(2):
                nc.tensor.matmul(
                    ps[:, :T],
                    peT_sb[:, kc, 1 + c * 128: 1 + (c + 1) * 128],
                    wfus_sb[:, m * 2 + kc, :],
                    start=(kc == 0), stop=(kc == 1 and m != 0),
                )
            if m == 0:
                nc.tensor.matmul(ps[:, :T], ones_row[:], bfus_sb[:],
                                 start=False, stop=True)
            nc.scalar.activation(ob[:, c, m % 2, :], ps[:, :T], AF.Copy)
    nc.sync.dma_start(pcatA[:].rearrange("(c p) m t -> p c m t", p=128), obA[:])
    nc.gpsimd.dma_start(pcatB[:].rearrange("(c p) m t -> p c m t", p=128), obB[:])

    # ---------------- phase 1: window loads + one-hot selection ----------------
    rel_tiles = [rel_p.tile([128, 2, S], BF, tag=f"rel{i}", name=f"rel_{i}") for i in range(IPC)]

    win_p = es.enter_context(tc.tile_pool(name="winp", bufs=6))
    from concourse.tile import add_dep_helper
    dma_hist = {nc.sync.engine: [], nc.gpsimd.engine: []}
    for i in range(IPC):
        rows = {}
        for k, eng in ((0, nc.sync), (1, nc.gpsimd)):
            hist = dma_hist[eng.engine]
            tmp = eng.alloc_register(f"woffr_{i}_{k}")
            ld = eng.reg_load(tmp, woff_sb[i:i + 1, k:k + 1])
            if len(hist) >= 12:
                add_dep_helper(ld.ins, hist[-12].ins, sync=False)
            rows[k] = eng.snap(tmp, donate=True, min_val=0, max_val=NROW - S)
        # winA[p, c, mslot, t]: row c*128+p of the 256-row window, table pair
        # A loaded then pair B accumulated on top via SDMA CCE add.
        winA = win_p.tile([128, 2, 2, S], F8, tag="winA", name=f"wA_{i}")
        dmi = nc.sync.dma_start(
            winA[:], pcatA[:][bass.ds(rows[0], S), :, :].rearrange("(c p) m t -> p c m t", p=128))
        dma_hist[nc.sync.engine].append(dmi)
        dmi = nc.gpsimd.dma_start(
            winA[:], pcatB[:][bass.ds(rows[1], S), :, :].rearrange("(c p) m t -> p c m t", p=128),
            accum_op=ALU.add)
        dma_hist[nc.gpsimd.engine].append(dmi)
        # one-hot selection on PE: rel_pre[t, j] = sum_u w[u, t] * onehot[u, j]
        ps = psum_p.tile([128, 512], F32, tag="ps", name=f"rp_{i}")
        nmm = 0
        for tpo in range(2):
            for mslot, oh in ((0, os_sb), (1, oe_sb)):
                nmm += 1
                nc.tensor.matmul(
                    ps[:, tpo * S:(tpo + 1) * S],
                    winA[:, :, mslot, tpo * 128:(tpo + 1) * 128],
                    oh[:],
                    start=(nmm in (1, 3)), stop=(nmm in (2, 4)),
                    perf_mode=mybir.MatmulPerfMode.DoubleRow,
                )
        if i % 2 == 0:
            nc.scalar.activation(rel_tiles[i][:], ps[:], AF.Relu)
        else:
            nc.vector.tensor_scalar(rel_tiles[i][:], ps[:], 0.0, None, ALU.max)
        emit_next_wload()

    if debug:
        r0 = add_p.tile([128, 2, S], F32, tag="dbgr", name="dbgrel")
        nc.vector.tensor_copy(r0[:], rel_tiles[0][:])
        nc.sync.dma_start(dbg["rel0"][:], r0[:])

    # persistent block-diag buffers (zeros survive across layers)
    g_blk = pers_p.tile([128, 2, 16 * IPB * 32], BF, tag="gblk", name="gblk")
    nc.vector.memset(g_blk[:], 0.0)
    qud = pers_p.tile([128, 2, IPC * 8], BF, tag="qud", name="qud")
    nc.vector.memset(qud[:], 0.0)

    # ---------------- phase 2: transformer layers ----------------
    curT_own = residT_sb  # [128, 2, IPC] fp32: own 64 tokens

    def layer_norm_T(src, name):
        mean_ps = psrow_p.tile([1, IPC], F32, tag="psr", name=f"mn_{name}")
        for c in range(2):
            nc.tensor.matmul(mean_ps[:], ones_col[:], src[:, c, :], start=(c == 0), stop=(c == 1))
        mean_sb = stat_p.tile([1, IPC], F32, tag="strow", name=f"mns_{name}")
        nc.vector.tensor_scalar_mul(mean_sb[:], mean_ps[:], 1.0 / T)
        mb_ps = psum_p.tile([128, 512], F32, tag="ps", name=f"mb_{name}")
        nc.tensor.matmul(mb_ps[:, :IPC], ones_row[:], mean_sb[:], start=True, stop=True)
        ym = work_p.tile([128, 2, IPC], F32, tag="ym", name=f"ym_{name}")
        ysq = work_p.tile([128, IPC], F32, tag="ysq", name=f"ysq_{name}")
        var_ps = psrow_p.tile([1, IPC], F32, tag="psr", name=f"vr_{name}")
        for c in range(2):
            nc.vector.tensor_sub(ym[:, c, :], src[:, c, :], mb_ps[:, :IPC])
        for c in range(2):
            nc.vector.tensor_mul(ysq[:], ym[:, c, :], ym[:, c, :])
            nc.tensor.matmul(var_ps[:], ones_col[:], ysq[:], start=(c == 0), stop=(c == 1))
        var_sb = stat_p.tile([1, IPC], F32, tag="strow", name=f"vrs_{name}")
        nc.vector.tensor_scalar(var_sb[:], var_ps[:], 1.0 / T, EPS, ALU.mult, ALU.add)
        rstd = stat_p.tile([1, IPC], F32, tag="strow", name=f"rs_{name}")
        nc.vector.reciprocal(rstd[:], var_sb[:])
        nc.scalar.activation(rstd[:], rstd[:], AF.Sqrt)
        rb_ps = psum_p.tile([128, 512], F32, tag="ps", name=f"rb_{name}")
        nc.tensor.matmul(rb_ps[:, :IPC], ones_row[:], rstd[:], start=True, stop=True)
        out = work_p.tile([128, 2, IPC], F32, tag=f"lnout_{name}", name=f"lno_{name}")
        for c in range(2):
            nc.vector.tensor_mul(out[:, c, :], ym[:, c, :], rb_ps[:, :IPC])
        return out

    for l in range(L):
        ownT_bf = work_p.tile([128, 2, IPC], BF, tag="ownbf", name=f"ownbf_{l}")
        nc.scalar.activation(ownT_bf[:], curT_own[:], AF.Copy)
        if l == 0:
            # token c of feature-chunk fc lives at xT_sb[:, fc, c]
            rhs_k = lambda c: xT_sb[:, c, :]
            stat_v = lambda c, jc: xT_sb[:, c, jc * 128:(jc + 1) * 128]
        else:
            rhs_k = lambda c: nxt[:, c, :]
            stat_v = lambda c, jc: nxt[:, c, jc * 128:(jc + 1) * 128]

        wq_sb, wk_sb, wv_sb = W["wq", l], W["wk", l], W["wv", l]
        wrT_sb, w1_sb, w2_sb = W["wrT", l], W["w1", l], W["w2", l]
        bk_sb, bv_sb, bqu_sb = W["bk", l], W["bv", l], W["bqu", l]
        bqv_sb, b1_sb, b2_sb = W["bqv", l], W["b1", l], W["b2", l]

        # ---- qu_T / qv_T [128, 2, IPC] bf16 ----
        quT = work_p.tile([128, 2, IPC], BF, tag="quT", name=f"quT_{l}")
        qvT = work_p.tile([128, 2, IPC], BF, tag="qvT", name=f"qvT_{l}")
        for po in range(2):
            ps = psum_p.tile([128, 512], F32, tag="ps", name=f"qps_{l}_{po}")
            for c in range(2):
                nc.tensor.matmul(ps[:, :IPC], wslice(wq_sb, c, po), ownT_bf[:, c, :], start=(c == 0), stop=(c == 1))
            nc.scalar.activation(quT[:, po, :], ps[:, :IPC], AF.Identity, bias=bqu_sb[:, po, :])
            nc.scalar.activation(qvT[:, po, :], ps[:, :IPC], AF.Identity, bias=bqv_sb[:, po, :])
        for h in range(H):
            hc, hp = divmod(h * D, 128)
            dstq = qud[:, hc, :].rearrange("p (i h) -> p i h", h=8)
            nc.vector.tensor_copy(dstq[hp:hp + D, :, h], quT[hp:hp + D, hc, :])

        # ---- gT -> block-diag g_blk ----
        for h in range(H):
            hc, hp = divmod(h * D, 128)
            for tp in range(2):
                ps = psum_p.tile([128, 512], F32, tag="ps", name=f"gps_{l}_{h}_{tp}")
                nc.tensor.matmul(
                    ps[:, :IPC], wrT_sb[hp:hp + D, hc, tp * 128:(tp + 1) * 128],
                    qvT[hp:hp + D, hc, :], start=True, stop=True,
                    tile_position=(hp, 0),
                )
                srcv = ps[:, :IPC].rearrange("p (s i) -> p s i", i=IPB)
                dstv = g_blk[:, tp, :].rearrange("p (s i c) -> p s i c", i=IPB, c=32)
                for ip in range(IPB):
                    nc.vector.tensor_copy(dstv[:, :, ip, 8 * ip + h], srcv[:, :, ip])

        # ---- k_T [128, 2, S] bf16 (needs all gathered tokens) ----
        kT = work_p.tile([128, 2, S], BF, tag="kT", name=f"kT_{l}")
        for po in range(2):
            ps = psum_p.tile([128, 512], F32, tag="ps", name=f"kps_{l}_{po}")
            for c in range(2):
                nc.tensor.matmul(ps[:, :S], wslice(wk_sb, c, po), rhs_k(c), start=(c == 0), stop=(c == 1))
            nc.scalar.activation(kT[:, po, :], ps[:, :S], AF.Identity, bias=bk_sb[:, po, :])

        # ---- val [128, 2(jc), T] bf16 ----
        val = work_p.tile([128, 2, T], BF, tag="val", name=f"val_{l}")
        for jc in range(2):
            ps = psum_p.tile([128, 512], F32, tag="ps", name=f"vps_{l}_{jc}")
            for c in range(2):
                nc.tensor.matmul(ps[:, :T], stat_v(c, jc), wv_sb[:, c, :], start=(c == 0), stop=False)
            nc.tensor.matmul(ps[:, :T], ones_row[:], bv_sb[:], start=False, stop=True)
            nc.scalar.activation(val[:, jc, :], ps[:, :T], AF.Copy)

        # ---- per-group score / softmax / attention ----
        yT = work_p.tile([128, 2, IPC], F32, tag="yT", name=f"yT_{l}")
        for g in range(4):
            score = psum_p.tile([128, 512], F32, tag="ps", name=f"sc_{l}_{g}")
            for c in range(2):
                nc.tensor.matmul(score[:, :S], qud[:, c, g * 128:(g + 1) * 128], kT[:, c, :], start=(c == 0), stop=False, skip_group_check=True)
            nc.tensor.matmul(score[:, :S], ones_row[:], mask_sb[:], start=False, stop=False, skip_group_check=True)
            for sl in range(4):
                for ip in range(IPB):
                    i = 16 * g + 4 * sl + ip
                    blk = (4 * g + sl) * IPB + ip
                    for tcc in range(2):
                        nc.tensor.matmul(
                            score[32 * sl:32 * sl + 32, :S],
                            g_blk[:, tcc, :].rearrange("p (b c) -> p b c", c=32)[:, blk, :],
                            rel_tiles[i][:, tcc, :],
                            start=False, stop=(ip == IPB - 1 and tcc == 1),
                            tile_position=(0, 32 * sl), skip_group_check=True,
                        )
            # softmax over j
            mx = stat_p.tile([128, 1], F32, tag="st", name=f"mx_{l}_{g}")
            nc.vector.tensor_reduce(mx[:], score[:, :S], AX.X, ALU.max, negate=True)
            prob = prob_p.tile([128, S], BF, tag="prob", name=f"pr_{l}_{g}")
            sum_row = stat_p.tile([128, 1], F32, tag="st", name=f"sm_{l}_{g}")
            nc.scalar.activation(prob[:], score[:, :S], AF.Exp, bias=mx[:], accum_out=sum_row[:])
            rcp = stat_p.tile([128, 1], F32, tag="st", name=f"rc_{l}_{g}")
            nc.vector.reciprocal(rcp[:], sum_row[:])
            nc.vector.tensor_scalar_mul(prob[:], prob[:], rcp[:])
            if debug and l == 0 and g == 0:
                scf = add_p.tile([128, S], F32, tag="dbgsc", name="dbgsc")
                nc.vector.tensor_copy(scf[:], score[:, :S])
                nc.sync.dma_start(dbg["score0"][:], scf[:])
                prf = add_p.tile([128, S], F32, tag="dbgpr", name="dbgpr")
                nc.vector.tensor_copy(prf[:], prob[:])
                nc.sync.dma_start(dbg["prob0"][:], prf[:])
            # prob^T and attention
            attn_ps = psum_p.tile([128, 512], F32, tag="ps", name=f"at_{l}_{g}")
            for jc in range(2):
                pt_ps = psum_p.tile([128, 1024], BF, tag="ps", name=f"pt_{l}_{g}_{jc}")
                nc.tensor.transpose(pt_ps[:, :128], prob[:, jc * 128:(jc + 1) * 128], ident_bf[:])
                pt_sb = prob_p.tile([128, 128], BF, tag="probT", name=f"pts_{l}_{g}_{jc}")
                nc.scalar.activation(pt_sb[:], pt_ps[:, :128], AF.Copy)
                for h in range(H):
                    hm, tau = h % 4, h // 4
                    nc.tensor.matmul(
                        attn_ps[hm * 32:(hm + 1) * 32, tau * 16:(tau + 1) * 16],
                        val[:, jc, h * 32:(h + 1) * 32],
                        pt_sb[:].rearrange("p (q h) -> p q h", h=8)[:, :, h],
                        start=(jc == 0 and tau == 0), stop=(jc == 1 and tau == 1),
                        tile_position=(0, hm * 32), skip_group_check=True,
                    )
            if debug and l == 0 and g == 0:
                atf = add_p.tile([128, 32], F32, tag="dbgat", name="dbgat")
                nc.vector.tensor_copy(atf[:], attn_ps[:, :32])
                nc.sync.dma_start(dbg["attn0"][:], atf[:])
            for fc in range(2):
                nc.vector.tensor_add(
                    yT[:, fc, 16 * g:16 * g + 16],
                    attn_ps[:, fc * 16:(fc + 1) * 16],
                    curT_own[:, fc, 16 * g:16 * g + 16],
                )

        y = layer_norm_T(yT, f"l{l}a")
        if debug and l == 0:
            nc.sync.dma_start(dbg["y1"][:].rearrange("(c p) s -> p c s", p=128), y[:])
        y_bf = work_p.tile([128, 2, IPC], BF, tag="ybf", name=f"ybf_{l}")
        nc.scalar.activation(y_bf[:], y[:], AF.Copy)

        # ---- FFN ----
        h1 = work_p.tile([128, 8, IPC], BF, tag="h1", name=f"h1_{l}")
        for fo in range(8):
            ps = psum_p.tile([128, 512], F32, tag="ps", name=f"h1p_{l}_{fo}")
            for c in range(2):
                nc.tensor.matmul(ps[:, :IPC], w1_sb[:, c, fo * 128:(fo + 1) * 128], y_bf[:, c, :], start=(c == 0), stop=(c == 1))
            nc.scalar.activation(h1[:, fo, :], ps[:, :IPC], AF.Relu, bias=b1_sb[:, fo, :])
        zT = work_p.tile([128, 2, IPC], F32, tag="zT", name=f"zT_{l}")
        for po in range(2):
            ps = psum_p.tile([128, 512], F32, tag="ps", name=f"zp_{l}_{po}")
            for c in range(8):
                nc.tensor.matmul(ps[:, :IPC], w2_sb[:, c, po * 128:(po + 1) * 128], h1[:, c, :], start=(c == 0), stop=(c == 7))
            nc.scalar.activation(zT[:, po, :], ps[:, :IPC], AF.Relu, bias=b2_sb[:, po, :])
        z_res = work_p.tile([128, 2, IPC], F32, tag="zres", name=f"zres_{l}")
        for c in range(2):
            nc.vector.tensor_add(z_res[:, c, :], zT[:, c, :], y[:, c, :])
        outT = layer_norm_T(z_res, f"l{l}b")

        if l == 0:
            if debug:
                nc.sync.dma_start(dbg["out1"][:].rearrange("(c p) s -> p c s", p=128), outT[:])
            # bf16 shard, AllGather, one reload per feature chunk
            sh_bf = work_p.tile([128, 2, IPC], BF, tag="shbf", name="sh_bf")
            nc.scalar.activation(sh_bf[:], outT[:], AF.Copy)
            shard = dram_p.tile([2, 128, IPC], BF, tag="agsh", name="ag_shard")
            nc.sync.dma_start(shard[:].rearrange("f p s -> p f s"), sh_bf[:])
            ag = dram_p.tile([4, 2, 128, IPC], BF, tag="agout", name="ag_out")
            nc.gpsimd.collective_compute(
                "AllGather", mybir.AluOpType.bypass,
                replica_groups=[[0, 1, 2, 3], [4, 5, 6, 7]],
                ins=[shard[:]], outs=[ag[:]],
            )
            nxt = pers_p.tile([128, 2, S], BF, tag="cur2", name="cur2")
            for fc in range(2):
                [nc.sync, nc.scalar][fc].dma_start(
                    nxt[:, fc, :].rearrange("p (q k) -> p q k", k=IPC),
                    ag[:, fc, :, :].rearrange("q p k -> p q k"))
            curT_own = outT
        else:
            nc.sync.dma_start(io["outT_d"][:].rearrange("(c p) s -> p c s", p=128), outT[:])

    es.close()


def wslice(w_sb, c, po):
    """Column slice [po*128, po*128+128) of weight chunk c."""
    return w_sb[:, c, po * 128:(po + 1) * 128]


# ====================== host side ======================

_CACHE = {}


def _get_nc(debug=False):
    key = ("nc", debug)
    if key not in _CACHE:
        _CACHE[key] = build_nc(debug=debug)
    return _CACHE[key]


def make_inputs_for_core(core, x, pos_s, pos_e, real_lengths, lex_num, pe,
                         W_fus, b_fus, Wq, bq, Wk, bk, Wv, bv, Wr, br,
                         u, v, W1, b1, W2, b2):
    b = core // 4
    i0 = (core % 4) * IPC
    xb = np.asarray(x[b], np.float32)          # [S, T]
    ps_b = np.asarray(pos_s[b]).astype(np.int64)
    pe_b = np.asarray(pos_e[b]).astype(np.int64)

    # windows: table m window start row in pcat; one-hot matrices for j-selection.
    iidx = np.arange(i0, i0 + IPC)
    woff = np.empty((IPC, 2), np.int32)
    woff[:, 0] = ps_b[iidx]
    woff[:, 1] = pe_b[iidx]
    osmat = np.zeros((S, S), np.float32)
    osmat[255 - ps_b, np.arange(S)] = 1.0
    oemat = np.zeros((S, S), np.float32)
    oemat[255 - pe_b, np.arange(S)] = 1.0

    keylen = int(real_lengths[b]) + int(lex_num)
    maskrow = np.where(np.arange(S) < keylen, 0.0, -1e15).astype(np.float32)[None, :]

    bf = lambda a: np.ascontiguousarray(np.asarray(a, np.float32)).astype(BF16)
    col = lambda a: np.ascontiguousarray(np.asarray(a, np.float32).reshape(-1, 1))

    uflat = np.asarray(u, np.float32).reshape(L, T)
    vflat = np.asarray(v, np.float32).reshape(L, T)

    return {
        "xT": bf(xb.T),
        "residT": np.ascontiguousarray(xb[i0:i0 + IPC].T),
        "peT": bf(np.asarray(pe, np.float32).T),
        "wfus": bf(W_fus),
        "bfus": np.asarray(b_fus, np.float32).reshape(1, T),
        "maskrow": maskrow,
        "osmat": osmat.astype(FP8), "oemat": oemat.astype(FP8),
        "woff": woff,
        "wq": bf(Wq), "wk": bf(Wk), "wv": bf(Wv),
        "wrT": bf(np.asarray(Wr, np.float32).transpose(0, 2, 1)),
        "w1": bf(W1), "w2": bf(W2),
        "bk": np.asarray(bk, np.float32).reshape(L, T, 1),
        "bv": np.asarray(bv, np.float32).reshape(L, 1, T),
        "bqu": (np.asarray(bq, np.float32) + uflat).reshape(L, T, 1),
        "bqv": (np.asarray(bq, np.float32) + vflat).reshape(L, T, 1),
        "b1": np.asarray(b1, np.float32).reshape(L, FF, 1),
        "b2": np.asarray(b2, np.float32).reshape(L, T, 1),
    }


def kernel(**inputs):
    from concourse.bass_utils import run_bass_kernel_spmd

    nc = _get_nc(debug=False)
    in_maps = [make_inputs_for_core(c, **inputs) for c in range(NC)]
    res = run_bass_kernel_spmd(nc, in_maps, list(range(NC)))
    out = np.empty((B, S, T), np.float32)
    for c in range(NC):
        b = c // 4
        i0 = (c % 4) * IPC
        out[b, i0:i0 + IPC, :] = res.results[c]["outT"].T
    return out

